# revision 8
# baseline (speedup 1.0000x reference)
"""Trainium2 Bass kernel for nn_Actor (topk_masking).

Pure data parallel across 8 NeuronCores: batch 16384 -> 2048 per core.
All weights replicated; BatchNorm folded into W1/W2 on the host.

Layout strategy: activations live feature-major ([feat partitions, batch free])
so every linear layer is weight-stationary matmul with the batch as the moving
operand (fp32, 512-wide). Head logits are PE-transposed to [batch, 64] tiles
where softmax/argmax/log-prob run on DVE/ACT in logit space (no divisions):
  select = argmax of filter-masked logits; logp = max - m - ln(sum(f*e^(l-m))).
The selected move row is fetched from DRAM with an indirect gather keyed by
on-chip argmax results.
"""

import numpy as np

B_FULL = 16384
N_CORES = 8
EPS = 1e-5


# ---------------------------------------------------------------------------
# host-side preprocessing
# ---------------------------------------------------------------------------

def _fold_bn(W, b, g, be, m, v):
    rs = (1.0 / np.sqrt(v + np.float32(EPS))).astype(np.float32)
    scale = (rs * g).astype(np.float32)
    Wf = (W * scale[None, :]).astype(np.float32)
    bf = ((b - m) * scale + be).astype(np.float32)
    return Wf, bf


def _prep_host(embeddings, teams, move_matrices, params):
    p = {k: np.asarray(v) for k, v in params.items()}
    W1, b1 = _fold_bn(np.asarray(p["W1"]), p["b1"], p["g1"], p["be1"], p["m1"], p["v1"])
    W2, b2 = _fold_bn(np.asarray(p["W2"]), p["b2"], p["g2"], p["be2"], p["m2"], p["v2"])

    weights = {
        "w1": W1, "w2": W2,
        "ws1": np.asarray(p["Ws1"]), "ws2": np.asarray(p["Ws2"]),
        "wt1a": np.ascontiguousarray(p["Wt1"][:512]),
        "wt1b": np.ascontiguousarray(p["Wt1"][512:576]),
        "wt2": np.asarray(p["Wt2"]),
        "wp1a": np.ascontiguousarray(p["Wp1"][:512]),
        "wp1s": np.ascontiguousarray(p["Wp1"][512:576]),
        "wp1t": np.ascontiguousarray(p["Wp1"][576:640]),
        "wp2": np.asarray(p["Wp2"]),
        # biases reshaped [128, n_m] (partition-within-chunk, chunk)
        "b1": b1.reshape(4, 128).T.copy(),
        "b2": b2.reshape(4, 128).T.copy(),
        "bs1": np.asarray(p["bs1"]).reshape(4, 128).T.copy(),
        "bt1": np.asarray(p["bt1"]).reshape(4, 128).T.copy(),
        "bp1": np.asarray(p["bp1"]).reshape(4, 128).T.copy(),
        "bs2": np.asarray(p["bs2"]).reshape(64, 1).copy(),
        "bt2": np.asarray(p["bt2"]).reshape(64, 1).copy(),
        "bp2": np.asarray(p["bp2"]).reshape(4, 1).copy(),
    }
    for k, v in weights.items():
        weights[k] = np.ascontiguousarray(v, dtype=np.float32)

    consts = {
        "ident": np.eye(128, dtype=np.float32),
        "iota_f": np.tile(np.arange(64, dtype=np.float32), (128, 1)),
        "wiota_f": np.tile(64.0 - np.arange(64, dtype=np.float32), (128, 1)),
        "iota_p64": (np.arange(128, dtype=np.float32) * 64.0).reshape(128, 1),
    }

    x_cat = np.concatenate(
        [np.asarray(embeddings, np.float32), np.asarray(teams, np.float32)], axis=1
    )  # [B_FULL, 512]
    mm = np.asarray(move_matrices)
    assert mm.dtype == np.int32

    Bc = B_FULL // N_CORES
    in_maps = []
    for c in range(N_CORES):
        sl = slice(c * Bc, (c + 1) * Bc)
        im = {
            "xT": np.ascontiguousarray(x_cat[sl].T),            # [512, Bc]
            "mmt": np.ascontiguousarray(mm[sl]).reshape(Bc * 64, 64),  # [Bc*64, 64]
        }
        im.update(weights)
        im.update(consts)
        in_maps.append(im)
    return in_maps


# ---------------------------------------------------------------------------
# walrus wait-cap workaround (this walrus accepts 1 wait per instruction,
# 2 on EventSemaphore; Tile freely packs more)
# ---------------------------------------------------------------------------

def _make_fixed_tile_context():
    import concourse.mybir as mybir
    import concourse.tile as tile

    class FixedTileContext(tile.TileContext):
        def _drain_and_barrier(self, tick_clock, wait_clock):
            nc = self.nc
            probe = nc.sync.nop()
            wait_clock.add_sem_waits(
                probe.ins, tile.ScopedClock({None: tick_clock.global_clock})
            )
            si = probe.ins.sync_info
            if si is not None and len(si.on_wait) > 1:
                waits = list(si.on_wait)
                si.on_wait = [waits[0]]
                for w in waits[1:]:
                    extra = nc.sync.nop()
                    extra.ins.sync_info = mybir.SyncInfo(on_wait=[w], on_update=[])
            nc.sync.drain()
            nc.all_engine_barrier()
            assert self.sems is not None
            popped = nc._tile_sem_poison_stack.pop()
            assert popped is self._sem_poison
            nc.clear_and_free_semaphores(list(self.sems.allocated().values()))
            nc.all_engine_barrier()

    return FixedTileContext


def _legalize_waits(nc):
    import concourse.mybir as mybir

    counter = [0]
    for f in nc.m.functions:
        blocks = f.blocks
        for bi in range(len(blocks)):
            bb = blocks[bi]
            new_insts = []
            changed = False
            for inst in bb.instructions:
                si = inst.sync_info
                cap = 2 if isinstance(inst, mybir.InstEventSemaphore) else 1
                if si is not None and len(si.on_wait) > cap:
                    waits = list(si.on_wait)
                    for w in waits[: len(waits) - cap]:
                        nop = mybir.InstNoOp(
                            name=f"I-wsplit-{counter[0]}", ins=[], outs=[]
                        )
                        counter[0] += 1
                        nop.engine = inst.engine
                        nop.sync_info = mybir.SyncInfo(on_wait=[w], on_update=[])
                        new_insts.append(nop)
                    si.on_wait = waits[len(waits) - cap:]
                    changed = True
                new_insts.append(inst)
            if changed:
                nb = mybir.BasicBlock(name=bb.name, instructions=new_insts)
                nb.IsExit = bb.IsExit
                nb.IsLoopEntry = bb.IsLoopEntry
                nb.IsPredicated = bb.IsPredicated
                blocks[bi] = nb


# ---------------------------------------------------------------------------
# device program
# ---------------------------------------------------------------------------

def build_program(Bc=B_FULL // N_CORES, BT=512, legalize=True):
    """Build the per-core Bass program. Bc = rows per core, BT = moving tile."""
    import concourse.bass as bass
    import concourse.mybir as mybir
    from concourse.bass import IndirectOffsetOnAxis

    f32 = mybir.dt.float32
    i32 = mybir.dt.int32
    AF = mybir.ActivationFunctionType
    ALU = mybir.AluOpType

    NBT = Bc // BT      # moving tiles per core
    NT = Bc // 128      # 128-row tiles per core
    TPB = BT // 128     # 128-row tiles per moving tile

    FixedTileContext = _make_fixed_tile_context()

    nc = bass.Bass("TRN2", target_bir_lowering=False, debug=False,
                   enable_asserts=True, num_devices=N_CORES)

    def din(name, shape, dt=f32):
        return nc.dram_tensor(name, shape, dt, kind="ExternalInput").ap()

    xT = din("xT", [512, Bc])
    mmt = din("mmt", [Bc * 64, 64], i32)
    w1 = din("w1", [512, 512]); w2 = din("w2", [512, 512])
    ws1 = din("ws1", [512, 512]); ws2 = din("ws2", [512, 64])
    wt1a = din("wt1a", [512, 512]); wt1b = din("wt1b", [64, 512])
    wt2 = din("wt2", [512, 64])
    wp1a = din("wp1a", [512, 512]); wp1s = din("wp1s", [64, 512])
    wp1t = din("wp1t", [64, 512]); wp2 = din("wp2", [512, 4])
    b1 = din("b1", [128, 4]); b2 = din("b2", [128, 4])
    bs1 = din("bs1", [128, 4]); bt1 = din("bt1", [128, 4]); bp1 = din("bp1", [128, 4])
    bs2 = din("bs2", [64, 1]); bt2 = din("bt2", [64, 1]); bp2 = din("bp2", [4, 1])
    ident = din("ident", [128, 128])
    iota_f = din("iota_f", [128, 64])
    wiota_f = din("wiota_f", [128, 64])
    iota_p64 = din("iota_p64", [128, 1])

    sel_out = nc.dram_tensor("sel_out", [Bc, 1], i32, kind="ExternalOutput").ap()
    tgt_out = nc.dram_tensor("tgt_out", [Bc, 1], i32, kind="ExternalOutput").ap()
    pro_out = nc.dram_tensor("pro_out", [Bc, 1], i32, kind="ExternalOutput").ap()
    logp_out = nc.dram_tensor("logp_out", [Bc, 1], f32, kind="ExternalOutput").ap()

    with FixedTileContext(nc) as tc:
        import contextlib
        ctx = contextlib.ExitStack()
        with ctx:
            wpool = ctx.enter_context(tc.tile_pool(name="wpool", bufs=1))
            actp = ctx.enter_context(tc.tile_pool(name="actp", bufs=8))
            projp = ctx.enter_context(tc.tile_pool(name="projp", bufs=1))
            movep = ctx.enter_context(tc.tile_pool(name="movep", bufs=2))
            filtp = ctx.enter_context(tc.tile_pool(name="filtp", bufs=16))
            slbp = ctx.enter_context(tc.tile_pool(name="slbp", bufs=2))
            ohp = ctx.enter_context(tc.tile_pool(name="ohp", bufs=1))
            rowp = ctx.enter_context(tc.tile_pool(name="rowp", bufs=1))
            ephp = ctx.enter_context(tc.tile_pool(name="ephp", bufs=2))
            stgp = ctx.enter_context(tc.tile_pool(name="stgp", bufs=1))
            wstr = ctx.enter_context(tc.tile_pool(name="wstr", bufs=16))
            pmm = ctx.enter_context(tc.tile_pool(name="pmm", bufs=4, space="PSUM"))
            phead = ctx.enter_context(tc.tile_pool(name="phead", bufs=2, space="PSUM"))
            ptr = ctx.enter_context(tc.tile_pool(name="ptr", bufs=2, space="PSUM"))

            # ---- weights / constants into SBUF ----
            def load_w_chunks(ap, K, M, name):
                """DRAM [K, M] -> chunk tiles [128, <=128] lists [k][m]."""
                nk = (K + 127) // 128
                nm = (M + 127) // 128
                out = []
                for k in range(nk):
                    row = []
                    kp = min(128, K - k * 128)
                    for m in range(nm):
                        mp = min(128, M - m * 128)
                        t = wpool.tile([kp, mp], f32, name=f"{name}_{k}_{m}")
                        nc.sync.dma_start(out=t[:], in_=ap[k*128:k*128+kp, m*128:m*128+mp])
                        row.append(t)
                    out.append(row)
                return out

            cw1 = load_w_chunks(w1, 512, 512, "w1")
            cw2 = load_w_chunks(w2, 512, 512, "w2")
            cws1 = load_w_chunks(ws1, 512, 512, "ws1")
            cws2 = load_w_chunks(ws2, 512, 64, "ws2")

            cwt1b = load_w_chunks(wt1b, 64, 512, "wt1b")[0]   # [m] of [64,128]
            cwt2 = load_w_chunks(wt2, 512, 64, "wt2")

            cwp1s = load_w_chunks(wp1s, 64, 512, "wp1s")[0]
            cwp1t = load_w_chunks(wp1t, 64, 512, "wp1t")[0]
            cwp2 = load_w_chunks(wp2, 512, 4, "wp2")

            def stream_w(dram_ap):
                def f(k, m):
                    t = wstr.tile([128, 128], f32, name="wst", tag="wst")
                    nc.sync.dma_start(
                        out=t[:],
                        in_=dram_ap[k*128:(k+1)*128, m*128:(m+1)*128])
                    return t[:]
                return f

            def load_plain(ap, shape, name):
                t = wpool.tile(shape, f32, name=name)
                nc.sync.dma_start(out=t[:], in_=ap[:])
                return t

            tb1 = load_plain(b1, [128, 4], "tb1")
            tb2 = load_plain(b2, [128, 4], "tb2")
            tbs1 = load_plain(bs1, [128, 4], "tbs1")
            tbt1 = load_plain(bt1, [128, 4], "tbt1")
            tbp1 = load_plain(bp1, [128, 4], "tbp1")
            tbs2 = load_plain(bs2, [64, 1], "tbs2")
            tbt2 = load_plain(bt2, [64, 1], "tbt2")
            tbp2 = load_plain(bp2, [4, 1], "tbp2")
            tid = load_plain(ident, [128, 128], "tid")
            tiota = load_plain(iota_f, [128, 64], "tiota")
            twiota = load_plain(wiota_f, [128, 64], "twiota")
            tiop64 = load_plain(iota_p64, [128, 1], "tiop64")

            tnegbig = wpool.tile([128, 64], f32, name="tnegbig")
            nc.vector.memset(tnegbig[:], -1e30)
            tc64 = wpool.tile([128, 1], f32, name="tc64")
            nc.vector.memset(tc64[:], 64.0)

            # ---- x input ----
            xc = []
            for k in range(4):
                t = actp.tile([128, Bc], f32, name=f"x{k}", tag="act")
                nc.sync.dma_start(out=t[:], in_=xT[k*128:(k+1)*128, :])
                xc.append(t)

            # ---- move-matrix reduction stream (independent of trunk) ----
            filt = []
            for t in range(NT):
                mv = movep.tile([128, 2048], i32, name=f"mv{t}", tag="mv")
                mv2 = movep.tile([128, 2048], i32, name=f"mv2{t}", tag="mv")
                src = mmt.rearrange("(t p s) j -> t p (s j)", t=NT, p=128)
                nc.sync.dma_start(out=mv[:], in_=src[t, :, 0:2048])
                nc.sync.dma_start(out=mv2[:], in_=src[t, :, 2048:4096])
                s_a = ephp.tile([128, 32], i32, name="s_a", tag="s_a")
                s_b = ephp.tile([128, 32], i32, name="s_b", tag="s_b")
                with nc.allow_low_precision(reason="int32 sums of values <= 128"):
                    nc.vector.reduce_sum(s_a[:],
                                         mv[:].rearrange("p (s j) -> p s j", j=64),
                                         axis=mybir.AxisListType.X)
                    nc.vector.reduce_sum(s_b[:],
                                         mv2[:].rearrange("p (s j) -> p s j", j=64),
                                         axis=mybir.AxisListType.X)
                ft = filtp.tile([128, 64], f32, name=f"ft{t}", tag="ft")
                nc.vector.tensor_scalar(out=ft[:, 0:32], in0=s_a[:], scalar1=0,
                                        scalar2=None, op0=ALU.is_gt)
                nc.vector.tensor_scalar(out=ft[:, 32:64], in0=s_b[:], scalar1=0,
                                        scalar2=None, op0=ALU.is_gt)
                filt.append(ft)

            # ---- dense layer helper ----
            def dense(in_fn, w_for, nk, bias_fn, out_fn, relu, mp=128, nm=4,
                      tag="pmm", pool=None):
                """for b, m: psum = sum_k w[k][m].T @ in[k][:, bcols]; act->out."""
                pool = pool or pmm
                for b in range(NBT):
                    cols = slice(b * BT, (b + 1) * BT)
                    for m in range(nm):
                        ps = pool.tile([mp, BT], f32, name=f"ps_{tag}", tag=tag,
                                       space="PSUM")
                        for k in range(nk):
                            nc.tensor.matmul(ps[:], w_for(k, m), in_fn(k, b, cols),
                                             start=(k == 0), stop=(k == nk - 1))
                        nc.scalar.activation(
                            out_fn(m, b, cols), ps[:],
                            AF.Relu if relu else AF.Identity,
                            bias=bias_fn(m), scale=1.0)

            def mk_acts(name):
                return [actp.tile([128, Bc], f32, name=f"{name}{m}", tag="act")
                        for m in range(4)]

            # trunk: L1, L2
            hc = mk_acts("h")
            dense(lambda k, b, c: xc[k][:, c], lambda k, m: cw1[k][m][:], 4,
                  lambda m: tb1[:, m:m+1], lambda m, b, c: hc[m][:, c], True)
            projc = [projp.tile([128, Bc], f32, name=f"proj{m}") for m in range(4)]
            dense(lambda k, b, c: hc[k][:, c], lambda k, m: cw2[k][m][:], 4,
                  lambda m: tb2[:, m:m+1], lambda m, b, c: projc[m][:, c], True)

            # select head trunk: S1, S2
            s1c = mk_acts("s1")
            dense(lambda k, b, c: projc[k][:, c], lambda k, m: cws1[k][m][:], 4,
                  lambda m: tbs1[:, m:m+1], lambda m, b, c: s1c[m][:, c], True)
            slc = [slbp.tile([64, BT], f32, name=f"sl{b}", tag="slb")
                   for b in range(NBT)]
            dense(lambda k, b, c: s1c[k][:, c], lambda k, m: cws2[k][0][:], 4,
                  lambda m: tbs2[:], lambda m, b, c: slc[b][:], False,
                  mp=64, nm=1, tag="phead", pool=phead)

            # per-128-row-tile head state
            ohsT = [ohp.tile([64, BT], f32, name=f"ohsT{b}") for b in range(NBT)]
            ohtT = [ohp.tile([64, BT], f32, name=f"ohtT{b}") for b in range(NBT)]
            rowf = [rowp.tile([128, 64], f32, name=f"rowf{t}") for t in range(NT)]
            st_sel = stgp.tile([128, NT], f32, name="st_sel")
            st_tgt = stgp.tile([128, NT], f32, name="st_tgt")
            st_pro = stgp.tile([128, NT], f32, name="st_pro")
            st_logp = stgp.tile([128, NT], f32, name="st_logp")

            def transpose_to(pout_shape, in_ap, kdim):
                tp = ptr.tile(pout_shape, f32, name="tp", tag="tp", space="PSUM")
                nc.tensor.matmul(tp[:], in_ap, tid[0:kdim, 0:kdim],
                                 is_transpose=True, start=True, stop=True)
                return tp

            def argmax64(fl, width, name):
                """first-occurrence argmax along free axis -> ([128,1] idx f32,
                [128,1] max f32)."""
                fm = ephp.tile([128, 1], f32, name=f"fm_{name}", tag=f"fm_{name}")
                nc.vector.tensor_reduce(fm[:], fl[:], axis=mybir.AxisListType.X,
                                        op=ALU.max)
                eq = ephp.tile([128, width], f32, name=f"eq_{name}", tag=f"eq_{name}")
                nc.vector.tensor_scalar(out=eq[:], in0=fl[:], scalar1=fm[:],
                                        scalar2=None, op0=ALU.is_equal)
                sc = ephp.tile([128, width], f32, name=f"sc_{name}", tag=f"sc_{name}")
                nc.vector.tensor_tensor(out=sc[:], in0=eq[:],
                                        in1=twiota[:, 0:width], op=ALU.mult)
                ms = ephp.tile([128, 1], f32, name=f"ms_{name}", tag=f"ms_{name}")
                nc.vector.tensor_reduce(ms[:], sc[:], axis=mybir.AxisListType.X,
                                        op=ALU.max)
                idx = ephp.tile([128, 1], f32, name=f"idx_{name}", tag=f"idx_{name}")
                nc.vector.scalar_tensor_tensor(out=idx[:], in0=ms[:], scalar=-1.0,
                                               in1=tc64[:], op0=ALU.mult, op1=ALU.add)
                return idx, fm

            def logits_bt(chunk_ap, t, width, name):
                """PE-transpose head logits [width, 128cols] -> SBUF [128, width]."""
                col = (t % TPB) * 128
                tp = transpose_to([128, width], chunk_ap[:, col:col+128], width)
                lb = ephp.tile([128, width], f32, name=f"lb_{name}", tag=f"lb_{name}")
                nc.vector.tensor_copy(lb[:], tp[:])
                return lb

            def softmax_logp(lb, weight_ap, width, name):
                """-> (negm [128,1], lnS [128,1]); S = sum(weight * e^(l - m))."""
                negm = ephp.tile([128, 1], f32, name=f"nm_{name}", tag=f"nm_{name}")
                nc.vector.tensor_reduce(negm[:], lb[:], axis=mybir.AxisListType.X,
                                        op=ALU.max, negate=True)
                ex = ephp.tile([128, width], f32, name=f"ex_{name}", tag=f"ex_{name}")
                nc.scalar.activation(ex[:], lb[:], AF.Exp, bias=negm[:], scale=1.0)
                sv = ephp.tile([128, 1], f32, name=f"sv_{name}", tag=f"sv_{name}")
                scr = ephp.tile([128, width], f32, name=f"scr_{name}",
                                tag=f"scr_{name}")
                nc.vector.tensor_tensor(out=scr[:], in0=ex[:], in1=weight_ap,
                                        op=ALU.mult)
                nc.vector.reduce_sum(sv[:], scr[:], axis=mybir.AxisListType.X)
                lns = ephp.tile([128, 1], f32, name=f"ls_{name}", tag=f"ls_{name}")
                nc.scalar.activation(lns[:], sv[:], AF.Ln)
                return negm, lns

            # ---- select heads + gather ----
            for t in range(NT):
                b = t // TPB
                slb = logits_bt(slc[b], t, 64, "s")
                # masked logits
                fl = ephp.tile([128, 64], f32, name="fl_s", tag="fl_s")
                nc.vector.tensor_copy(fl[:], tnegbig[:])
                nc.vector.copy_predicated(fl[:], filt[t][:].bitcast(i32), slb[:])
                negm, lns = softmax_logp(slb, filt[t][:], 64, "s")
                idx, fm = argmax64(fl, 64, "s")
                nc.vector.tensor_copy(st_sel[:, t:t+1], idx[:])
                # slp = fm - m - lnS
                a0 = ephp.tile([128, 1], f32, name="a0_s", tag="a0_s")
                nc.vector.tensor_tensor(out=a0[:], in0=fm[:], in1=negm[:], op=ALU.add)
                nc.vector.tensor_tensor(out=st_logp[:, t:t+1], in0=a0[:], in1=lns[:],
                                        op=ALU.subtract)
                # one-hot(select) -> transpose into ohsT[b][:, tcols]
                ohs = ephp.tile([128, 64], f32, name="ohs", tag="ohs")
                nc.vector.tensor_scalar(out=ohs[:], in0=tiota[:], scalar1=idx[:],
                                        scalar2=None, op0=ALU.is_equal)
                tp = transpose_to([64, 128], ohs[:], 128)
                col = (t % TPB) * 128
                nc.vector.tensor_copy(ohsT[b][:, col:col+128], tp[:])
                # gather move row: row index = 64*(128*t + p) + select
                gi = ephp.tile([128, 1], f32, name="gi", tag="gi")
                nc.vector.scalar_tensor_tensor(out=gi[:], in0=tiop64[:],
                                               scalar=float(t * 8192), in1=idx[:],
                                               op0=ALU.add, op1=ALU.add)
                gii = ephp.tile([128, 1], i32, name="gii", tag="gii")
                nc.vector.tensor_copy(gii[:], gi[:])
                rowi = ephp.tile([128, 64], i32, name="rowi", tag="rowi")
                nc.gpsimd.indirect_dma_start(
                    out=rowi[:], out_offset=None, in_=mmt[:],
                    in_offset=IndirectOffsetOnAxis(ap=gii[:, :1], axis=0))
                nc.vector.tensor_copy(rowf[t][:], rowi[:])

            # ---- target trunk: T1 (proj + ohsT), T2 ----
            t1c = mk_acts("t1")
            def t1_in(k, b, c):
                return projc[k][:, c] if k < 4 else ohsT[b][:]
            swt1a = stream_w(wt1a)
            def t1_w(k, m):
                return swt1a(k, m) if k < 4 else cwt1b[m][:]
            dense(t1_in, t1_w, 5, lambda m: tbt1[:, m:m+1],
                  lambda m, b, c: t1c[m][:, c], True)
            tlc = [slbp.tile([64, BT], f32, name=f"tl{b}", tag="tlb")
                   for b in range(NBT)]
            dense(lambda k, b, c: t1c[k][:, c], lambda k, m: cwt2[k][0][:], 4,
                  lambda m: tbt2[:], lambda m, b, c: tlc[b][:], False,
                  mp=64, nm=1, tag="phead", pool=phead)

            # ---- target heads ----
            for t in range(NT):
                b = t // TPB
                tlb = logits_bt(tlc[b], t, 64, "t")
                # fl = where(row>0, tl + ln(max(row,0.5)), -big)
                rmask = ephp.tile([128, 64], i32, name="rmask", tag="rmask")
                nc.vector.tensor_scalar(out=rmask[:], in0=rowf[t][:], scalar1=0.0,
                                        scalar2=None, op0=ALU.is_gt)
                rm = ephp.tile([128, 64], f32, name="rm", tag="rm")
                nc.vector.tensor_scalar_max(rm[:], rowf[t][:], 0.5)
                lr = ephp.tile([128, 64], f32, name="lr", tag="lr")
                nc.scalar.activation(lr[:], rm[:], AF.Ln)
                fl0 = ephp.tile([128, 64], f32, name="fl0", tag="fl0")
                nc.vector.tensor_tensor(out=fl0[:], in0=tlb[:], in1=lr[:], op=ALU.add)
                fl = ephp.tile([128, 64], f32, name="fl_t", tag="fl_t")
                nc.vector.tensor_copy(fl[:], tnegbig[:])
                nc.vector.copy_predicated(fl[:], rmask[:], fl0[:])
                negm, lns = softmax_logp(tlb, rowf[t][:], 64, "t")
                idx, fm = argmax64(fl, 64, "t")
                nc.vector.tensor_copy(st_tgt[:, t:t+1], idx[:])
                a0 = ephp.tile([128, 1], f32, name="a0_t", tag="a0_t")
                nc.vector.tensor_tensor(out=a0[:], in0=fm[:], in1=negm[:], op=ALU.add)
                a1 = ephp.tile([128, 1], f32, name="a1_t", tag="a1_t")
                nc.vector.tensor_tensor(out=a1[:], in0=a0[:], in1=lns[:],
                                        op=ALU.subtract)
                nc.vector.tensor_tensor(out=st_logp[:, t:t+1],
                                        in0=st_logp[:, t:t+1], in1=a1[:], op=ALU.add)
                oht = ephp.tile([128, 64], f32, name="oht", tag="oht")
                nc.vector.tensor_scalar(out=oht[:], in0=tiota[:], scalar1=idx[:],
                                        scalar2=None, op0=ALU.is_equal)
                tp = transpose_to([64, 128], oht[:], 128)
                col = (t % TPB) * 128
                nc.vector.tensor_copy(ohtT[b][:, col:col+128], tp[:])
                # pf = (row[target] == 2), via sum(row * oh_t)
                pv = ephp.tile([128, 1], f32, name="pv", tag="pv")
                scr2 = ephp.tile([128, 64], f32, name="scr2", tag="scr2")
                nc.vector.tensor_tensor(out=scr2[:], in0=rowf[t][:], in1=oht[:],
                                        op=ALU.mult)
                nc.vector.reduce_sum(pv[:], scr2[:], axis=mybir.AxisListType.X)
                pf = filtp.tile([128, 1], f32, name=f"pf{t}", tag="pf")
                nc.vector.tensor_scalar(out=pf[:], in0=pv[:], scalar1=2.0,
                                        scalar2=None, op0=ALU.is_equal)
                filt.append(pf)  # keep handle alive; indexed NT+t below

            # ---- promote trunk: P1 (proj + ohsT + ohtT), P2 ----
            p1c = mk_acts("p1")
            def p1_in(k, b, c):
                if k < 4:
                    return projc[k][:, c]
                return ohsT[b][:] if k == 4 else ohtT[b][:]
            swp1a = stream_w(wp1a)
            def p1_w(k, m):
                if k < 4:
                    return swp1a(k, m)
                return cwp1s[m][:] if k == 4 else cwp1t[m][:]
            dense(p1_in, p1_w, 6, lambda m: tbp1[:, m:m+1],
                  lambda m, b, c: p1c[m][:, c], True)
            plc = [slbp.tile([4, BT], f32, name=f"pl{b}", tag="plb")
                   for b in range(NBT)]
            dense(lambda k, b, c: p1c[k][:, c], lambda k, m: cwp2[k][0][:], 4,
                  lambda m: tbp2[:], lambda m, b, c: plc[b][:], False,
                  mp=4, nm=1, tag="phead", pool=phead)

            # ---- promote heads + output assembly ----
            for t in range(NT):
                b = t // TPB
                plb = logits_bt(plc[b], t, 4, "p")
                pf = filt[NT + t]
                # promote logp = -ln(sum e^(pl - max)) if pf else 0
                negm3 = ephp.tile([128, 1], f32, name="nm_p", tag="nm_p")
                nc.vector.tensor_reduce(negm3[:], plb[:], axis=mybir.AxisListType.X,
                                        op=ALU.max, negate=True)
                ex3 = ephp.tile([128, 4], f32, name="ex_p", tag="ex_p")
                s3 = ephp.tile([128, 1], f32, name="s3", tag="s3")
                nc.scalar.activation(ex3[:], plb[:], AF.Exp, bias=negm3[:],
                                     scale=1.0, accum_out=s3[:])
                ls3 = ephp.tile([128, 1], f32, name="ls3", tag="ls3")
                nc.scalar.activation(ls3[:], s3[:], AF.Ln)
                plp = ephp.tile([128, 1], f32, name="plp", tag="plp")
                nc.vector.scalar_tensor_tensor(out=plp[:], in0=ls3[:], scalar=-1.0,
                                               in1=pf[:], op0=ALU.mult, op1=ALU.mult)
                nc.vector.tensor_tensor(out=st_logp[:, t:t+1],
                                        in0=st_logp[:, t:t+1], in1=plp[:], op=ALU.add)
                idx, _fm = argmax64(plb, 4, "p")
                # promote = (idx+1)*pf - 1
                pr = ephp.tile([128, 1], f32, name="pr", tag="pr")
                nc.vector.scalar_tensor_tensor(out=pr[:], in0=idx[:], scalar=1.0,
                                               in1=pf[:], op0=ALU.add, op1=ALU.mult)
                nc.vector.tensor_scalar(out=st_pro[:, t:t+1], in0=pr[:], scalar1=1.0,
                                        scalar2=None, op0=ALU.subtract)

            # ---- cast + DMA outputs ----
            def emit_out(stage, dram, dt):
                cast = stgp.tile([128, NT], dt, name=f"cast_{dram.tensor.name}")
                nc.vector.tensor_copy(cast[:], stage[:])
                dst = dram.rearrange("(t p) one -> p t one", p=128)
                nc.sync.dma_start(out=dst[:, :, 0], in_=cast[:])

            emit_out(st_sel, sel_out, i32)
            emit_out(st_tgt, tgt_out, i32)
            emit_out(st_pro, pro_out, i32)
            emit_out(st_logp, logp_out, f32)

    if legalize:
        _legalize_waits(nc)
    return nc


# ---------------------------------------------------------------------------
# entry point
# ---------------------------------------------------------------------------

_CACHED = {}


def kernel(embeddings, teams, move_matrices, params):
    from concourse.bass_utils import run_bass_kernel_spmd

    in_maps = _prep_host(embeddings, teams, move_matrices, params)
    if "nc" not in _CACHED:
        _CACHED["nc"] = build_program()
    nc = _CACHED["nc"]
    res = run_bass_kernel_spmd(nc, in_maps, core_ids=list(range(N_CORES)))
    sel = np.concatenate([r["sel_out"] for r in res.results]).astype(np.int32)
    tgt = np.concatenate([r["tgt_out"] for r in res.results]).astype(np.int32)
    pro = np.concatenate([r["pro_out"] for r in res.results]).astype(np.int32)
    logp = np.concatenate([r["logp_out"] for r in res.results]).astype(np.float32)
    return sel, tgt, pro, logp
